# revision 6
# baseline (speedup 1.0000x reference)
"""Trainium2 Bass kernel for sliding-window GQA attention block (v4).

Reference computation (B=2, S=4096, DIM=1024, H=16 q-heads, KV=2 kv-heads,
D=64, W=256 window):
    q = x@Wq + bq ; k = x@Wk + bk ; v = x@Wv + bv        (GQA repeat kv x8)
    local attention: query t attends keys [t-128, t+128) (zero-padded edges,
    no 1/sqrt(d) scaling), softmax, out = probs@v
    y = out@Wo + bo
Sharding: 8 cores = batch(2) x seq-quarter(4); each core computes 1024 query
rows end-to-end from a 1280-row haloed x slice; bo added on host.

v4 structure (vs 146us v2 baseline):
  - epilogue constants are HOST inputs: bkm = outer(bk, ind), bvm =
    bv x ind, band masks mA/mB.  x halo columns are zero so projections
    are already zero there -> single tensor_add fuses bias + halo mask.
  - PSUM as [128,1024] bank-pairs; both kv-halves of a score j-chunk
    share a pair -> ONE exp ACTIVATE per chunk (the 352-cycle ACT fixed
    cost amortizes), one reciprocal per (mt,gg).
  - attnT k-tile order k = 4*gg + 2*half + c (host permutes Wo rows to
    match) makes the normalize a single 3D-AP STT per (mt,gg,e).
  - band masks on DVE (tensor_mul with host masks); gpsimd only issues
    DMA descriptors (its semaphore path is too slow for the p2 chain).
  - head: all projections (Q/K/V) run before the loop; Q m0-3 are
    emitted k-major so the PE streams behind the xT/wq DMAs, which are
    interleaved per-chunk across the three DMA queues.  The attention
    loop is then pure scores/PV/out-proj.
  - ACT exp table preloaded with a dummy activation at t=0.
"""

import functools
import numpy as np

B, S, DIM = 2, 4096, 1024
H, KV, D = 16, 2, 64
W, HW = 256, 128
NCORES = 8
QT = 4           # sequence quarters
T = S // QT      # 1024 query rows per core
TH = T + 2 * HW  # 1280 haloed rows
NU = TH // 128   # 10 key/value u-tiles
KVD = KV * D     # 128


@functools.lru_cache(maxsize=1)
def _build_nc():
    import concourse.bacc as bacc
    import concourse.tile as tile
    from concourse import mybir

    f32 = mybir.dt.float32
    bf16 = mybir.dt.bfloat16
    Exp = mybir.ActivationFunctionType.Exp
    Identity = mybir.ActivationFunctionType.Identity
    MUL = mybir.AluOpType.mult

    nc = bacc.Bacc("TRN2", target_bir_lowering=False, debug=False)

    xTp = nc.dram_tensor("xTp", [128, 8 * TH], bf16, kind="ExternalInput")
    wqp = nc.dram_tensor("wqp", [128, 8 * DIM], bf16, kind="ExternalInput")
    wkp = nc.dram_tensor("wkp", [128, 8 * KVD], bf16, kind="ExternalInput")
    wvp = nc.dram_tensor("wvp", [128, 8 * KVD], bf16, kind="ExternalInput")
    wop = nc.dram_tensor("wop", [128, 8 * DIM], bf16, kind="ExternalInput")
    bqc = nc.dram_tensor("bqc", [128, 8], f32, kind="ExternalInput")
    bkmp = nc.dram_tensor("bkmp", [128, TH], bf16, kind="ExternalInput")
    bvmp = nc.dram_tensor("bvmp", [128, TH], bf16, kind="ExternalInput")
    mAp = nc.dram_tensor("mAp", [128, 1024], bf16, kind="ExternalInput")
    mBp = nc.dram_tensor("mBp", [128, 1024], bf16, kind="ExternalInput")
    out = nc.dram_tensor("out", [T, DIM], bf16, kind="ExternalOutput")

    with tile.TileContext(nc) as tc:
        with tc.tile_pool(name="const", bufs=1) as const, \
             tc.tile_pool(name="w", bufs=1) as wpool, \
             tc.tile_pool(name="act", bufs=1) as actp, \
             tc.tile_pool(name="attn", bufs=2) as attnp, \
             tc.tile_pool(name="ps", bufs=2, space="PSUM") as ps:

            # ---- ACT exp-table preload at t=0 (overlaps the DMA phase)
            dum = const.tile([1, 2], f32, tag="dum")
            dumo = const.tile([1, 2], f32, tag="dumo")
            nc.vector.memset(dum, 0.0)
            nc.scalar.activation(out=dumo, in_=dum, func=Exp)

            # ---- SBUF tiles -----------------------------------------------
            xT_sb = wpool.tile([128, 8 * TH], bf16, tag="xT")
            wq_sb = wpool.tile([128, 8 * DIM], bf16, tag="wq")
            wk_sb = wpool.tile([128, 8 * KVD], bf16, tag="wk")
            wv_sb = wpool.tile([128, 8 * KVD], bf16, tag="wv")
            wo_sb = wpool.tile([128, 8 * DIM], bf16, tag="wo")
            bq_sb = const.tile([128, 8], f32, tag="bq")
            bkm_sb = const.tile([128, TH], bf16, tag="bkm")
            bvm_sb = const.tile([128, TH], bf16, tag="bvm")
            mA = const.tile([128, 1024], bf16, tag="mA")
            mB = const.tile([128, 1024], bf16, tag="mB")

            # ---- DMA schedule: wk/wv first (tiny, K/V never wait), then
            # xT/wq chunk-interleaved across the three DMA queues in the
            # order the k-major Q matmuls consume them; consts next; wo
            # (first needed ~45us in) last, split across queues.
            # sync and gpsimd queues run ~145GB/s, scalar only ~70GB/s:
            # xT/wq alternate on the two fast queues (chunk k of both lands
            # before chunk k+1), everything small on scalar, wo split last.
            sq, sc, gq = nc.sync, nc.scalar, nc.gpsimd
            for k in range(8):
                a, b = (sq, gq) if k % 2 == 0 else (gq, sq)
                a.dma_start(out=xT_sb[:, k * TH:(k + 1) * TH],
                            in_=xTp[:, k * TH:(k + 1) * TH])
                b.dma_start(out=wq_sb[:, k * DIM:(k + 1) * DIM],
                            in_=wqp[:, k * DIM:(k + 1) * DIM])
            sc.dma_start(out=wk_sb, in_=wkp[:, :])
            sc.dma_start(out=wv_sb, in_=wvp[:, :])
            sc.dma_start(out=bq_sb, in_=bqc[:, :])
            sc.dma_start(out=bkm_sb, in_=bkmp[:, :])
            sc.dma_start(out=bvm_sb, in_=bvmp[:, :])
            sc.dma_start(out=mA, in_=mAp[:, :])
            sc.dma_start(out=mB, in_=mBp[:, :])
            for k in range(8):
                q = (sq, gq, sc, sq, gq, sc, sq, gq)[k]
                q.dma_start(out=wo_sb[:, k * DIM:(k + 1) * DIM],
                            in_=wop[:, k * DIM:(k + 1) * DIM])

            # ---- activations / attention SBUF -----------------------------
            qT_sb = [actp.tile([128, 4 * T], bf16, tag=f"qT{g}", name=f"qT{g}")
                     for g in range(2)]
            kT_sb = actp.tile([128, TH], bf16, tag="kT")
            v_sb = actp.tile([128, NU * 256], bf16, tag="V")
            v_view = v_sb.rearrange("p (u g c) -> p u g c", u=NU, g=2)
            nc.vector.memset(v_view[:, :, :, 0:64], 1.0)
            bvm_v = bvm_sb.rearrange("p (u g d) -> p u g d", u=NU, g=2)
            attnT = actp.tile([128, 8 * T], bf16, tag="attnT")
            attnT_v = attnT.rearrange("p (k t) -> p k t", k=8)
            qvs = [qT_sb[g].rearrange("p (i t) -> p i t", i=4) for g in range(2)]

            # ---- Q projection: two m-tiles k-inner per call; the first two
            # calls' k-loops stream directly behind the xT/wq DMAs.  Bias
            # copy into qT: even m on ACT, odd m on DVE.
            def q_sub(ms, interleaved=False):
                prs = {m: ps.tile([128, 1024], f32, tag="SP", bufs=2,
                                  name=f"qp{m}") for m in ms}
                for k in range(8):
                    for m in ms:
                        for n in range(2):
                            nc.tensor.matmul(
                                out=prs[m][:, n * 512:(n + 1) * 512],
                                lhsT=wq_sb[:, k * DIM + m * 128:
                                           k * DIM + (m + 1) * 128],
                                rhs=xT_sb[:, k * TH + HW + n * 512:
                                          k * TH + HW + (n + 1) * 512],
                                start=(k == 0), stop=(k == 7))
                for m in ms:
                    dst = qT_sb[m // 4][:, (m % 4) * T:(m % 4) * T + 1024]
                    if m % 2 == 0:
                        nc.scalar.activation(out=dst, in_=prs[m],
                                             func=Identity,
                                             bias=bq_sb[:, m:m + 1], scale=1.0)
                    else:
                        nc.vector.tensor_scalar_add(out=dst, in0=prs[m],
                                                    scalar1=bq_sb[:, m:m + 1])

            # sg0+sg1 interleaved k-major: 4 PSUM pairs live, PE consumes
            # each (xT[k], wq[k]) chunk-pair with 16 matmuls as it lands.
            def q_head():
                prs = {m: ps.tile([128, 1024], f32,
                                  tag=("SP" if m < 2 else "OP"), bufs=2,
                                  name=f"qp{m}") for m in range(4)}
                for k in range(8):
                    for m in range(4):
                        for n in range(2):
                            nc.tensor.matmul(
                                out=prs[m][:, n * 512:(n + 1) * 512],
                                lhsT=wq_sb[:, k * DIM + m * 128:
                                           k * DIM + (m + 1) * 128],
                                rhs=xT_sb[:, k * TH + HW + n * 512:
                                          k * TH + HW + (n + 1) * 512],
                                start=(k == 0), stop=(k == 7))
                for m in range(4):
                    dst = qT_sb[0][:, m * T:m * T + 1024]
                    if m % 2 == 0:
                        nc.scalar.activation(out=dst, in_=prs[m],
                                             func=Identity,
                                             bias=bq_sb[:, m:m + 1], scale=1.0)
                    else:
                        nc.vector.tensor_scalar_add(out=dst, in0=prs[m],
                                                    scalar1=bq_sb[:, m:m + 1])

            def k_proj():
                kp01 = ps.tile([128, 1024], f32, tag="SP", bufs=2, name="kp01")
                kp2 = ps.tile([128, 1024], f32, tag="OP", bufs=2, name="kp2")
                for k in range(8):
                    for c in range(2):
                        nc.tensor.matmul(
                            out=kp01[:, c * 512:(c + 1) * 512],
                            lhsT=wk_sb[:, k * KVD:(k + 1) * KVD],
                            rhs=xT_sb[:, k * TH + c * 512:k * TH + (c + 1) * 512],
                            start=(k == 0), stop=(k == 7))
                    nc.tensor.matmul(
                        out=kp2[:, 0:256],
                        lhsT=wk_sb[:, k * KVD:(k + 1) * KVD],
                        rhs=xT_sb[:, k * TH + 1024:k * TH + 1280],
                        start=(k == 0), stop=(k == 7))
                nc.vector.tensor_add(out=kT_sb[:, 0:1024], in0=kp01,
                                     in1=bkm_sb[:, 0:1024])
                nc.vector.tensor_add(out=kT_sb[:, 1024:1280], in0=kp2[:, 0:256],
                                     in1=bkm_sb[:, 1024:1280])

            def v_proj(ut):
                vp = ps.tile([128, 1024], f32, tag="OP", bufs=2, name="vp")
                for k in range(8):
                    nc.tensor.matmul(
                        out=vp[:, 0:128],
                        lhsT=xT_sb[:, k * TH + ut * 128:k * TH + (ut + 1) * 128],
                        rhs=wv_sb[:, k * KVD:(k + 1) * KVD],
                        start=(k == 0), stop=(k == 7))
                nc.vector.tensor_add(
                    out=v_view[:, ut, :, 64:128],
                    in0=vp[:, 0:128].rearrange("p (g c) -> p g c", g=2),
                    in1=bvm_v[:, ut])

            # ---- scores j-chunk: both kv-halves into one PSUM pair, one
            # exp over [128,1024]; band mask (j=0/2) via DVE tensor_mul.
            def scores_pair(mt, gg, j):
                qcol = mt * 128
                sp = ps.tile([128, 1024], f32, tag="SP", bufs=2, name="sp")
                for h in range(2):
                    nc.tensor.matmul(
                        out=sp[:, h * 512:(h + 1) * 512],
                        lhsT=kT_sb[h * 64:(h + 1) * 64,
                                   qcol + j * 128:qcol + (j + 1) * 128],
                        rhs=qvs[gg][h * 64:(h + 1) * 64, :, qcol:qcol + 128],
                        start=True, stop=True,
                        tile_position=(64 * h, 0))
                p2 = attnp.tile([128, 1024], bf16, tag="p2", bufs=10, name="p2")
                nc.scalar.activation(out=p2, in_=sp, func=Exp)
                if j == 0:
                    nc.vector.tensor_mul(p2, p2, mA)
                elif j == 2:
                    nc.vector.tensor_mul(p2, p2, mB)
                return p2

            # ---- PV + normalize: probs@[1|V] per half into one output
            # pair; one recip [64,1024]; one STT per e writes the four
            # k'-tiles (k' = 4gg + 2h + c, contiguous) of attnT.
            def pv(mt, gg, p2s):
                qcol = mt * 128
                op = ps.tile([128, 1024], f32, tag="OP", bufs=2, name="op")
                for h in range(2):
                    for j in range(3):
                        nc.tensor.matmul(
                            out=op[:, h * 512:(h + 1) * 512],
                            lhsT=v_view[:, mt + j, h, :],
                            rhs=p2s[j][:, h * 512:(h + 1) * 512],
                            start=(j == 0), stop=(j == 2))
                rc = attnp.tile([64, 1024], f32, tag="rc", bufs=2, name="rc")
                nc.vector.reciprocal_approx_fast(out=rc, in_=op[0:64, :])
                # free dim of op[64:128] is (h, c, e, t); for fixed e the
                # (h, c) dims are stride 512/256 -> merge into one 4-wide
                # dim matching attnT k'-tiles 4gg..4gg+3 (stride T).
                num = op[64:128, :].rearrange("p (hc e t) -> p hc e t",
                                              hc=4, e=2)
                rcv = rc.rearrange("p (hc e t) -> p hc e t", hc=4, e=2)
                for e in range(2):
                    nc.vector.scalar_tensor_tensor(
                        out=attnT_v[64 * e:64 * e + 64, 4 * gg:4 * gg + 4,
                                    qcol:qcol + 128],
                        in0=num[:, :, e, :], scalar=1.0,
                        in1=rcv[:, :, e, :], op0=MUL, op1=MUL)

            # ---- out projection: one PSUM pair; gg0 k'-tiles (0-3) first
            # so the accumulation overlaps the second PV group's normalize.
            def oproj(mt):
                qcol = mt * 128
                o2 = ps.tile([128, 1024], f32, tag="OP", bufs=2, name="o2")
                for k in range(8):
                    for n in range(2):
                        nc.tensor.matmul(
                            out=o2[:, n * 512:(n + 1) * 512],
                            lhsT=attnT[:, k * T + qcol:k * T + qcol + 128],
                            rhs=wo_sb[:, k * DIM + n * 512:
                                      k * DIM + (n + 1) * 512],
                            start=(k == 0), stop=(k == 7))
                out_t = attnp.tile([128, DIM], bf16, tag="outt", bufs=2,
                                   name="out_t")
                nc.scalar.copy(out=out_t[:, 0:512], in_=o2[:, 0:512])
                nc.sync.dma_start(out=out[qcol:qcol + 128, 0:512],
                                  in_=out_t[:, 0:512])
                nc.vector.tensor_copy(out=out_t[:, 512:1024],
                                      in_=o2[:, 512:1024])
                nc.sync.dma_start(out=out[qcol:qcol + 128, 512:1024],
                                  in_=out_t[:, 512:1024])

            # ---- head: minimum projections for loop start: Q m0-3 k-major
            # behind the DMA stream, K, V 0-2.  Q m4-7 run inside iteration
            # 0 (before its gg1 scores); V 3-9 are per-iteration fillers.
            q_head()
            k_proj()
            for ut in range(3):
                v_proj(ut)

            # ---- attention loop: per iteration
            #   [sc(mt,0) x3] [fillers] [pv(mt-1,1)] [sc(mt,1) x3]
            #   [oproj(mt-1)] [pv(mt,0)]
            # so each gg's exp->mask->PV->recip->STT chain is covered by
            # 5-7us of independent PE work before its consumer runs.
            prev_p2g1 = None
            prev = None
            for mt in range(8):
                last = (mt == 7)
                if last:
                    g1 = [scores_pair(mt, 1, j) for j in range(3)]
                g0 = [scores_pair(mt, 0, j) for j in range(3)]
                if mt == 0:
                    q_sub((4, 5))
                    q_sub((6, 7))
                if 1 <= mt <= 7:
                    v_proj(mt + 2)
                if prev is not None:
                    pv(prev, 1, prev_p2g1)
                if not last:
                    g1 = [scores_pair(mt, 1, j) for j in range(3)]
                if prev is not None:
                    oproj(prev)
                pv(mt, 0, g0)
                prev_p2g1 = g1
                prev = mt
            pv(prev, 1, prev_p2g1)
            oproj(prev)

    nc.compile()
    return nc


def _host_prep(x, Wq, bq, Wk, bk, Wv, bv, Wo, bo):
    import ml_dtypes
    bf16 = ml_dtypes.bfloat16

    def fold8(a, width):
        # [1024, width] -> [128, 8*width] with chunk k at cols k*width
        return np.ascontiguousarray(
            a.reshape(8, 128, width).transpose(1, 0, 2).reshape(128, 8 * width))

    # permute Wq columns so qT m-tile holds head m on partitions 0-63 and
    # head m+8 on partitions 64-127 (row-packed score matmuls)
    idx = np.empty(DIM, dtype=np.int64)
    for m in range(8):
        for j in range(128):
            h = m if j < 64 else m + 8
            idx[m * 128 + j] = h * D + (j % 64)
    wq_p = fold8(np.ascontiguousarray(Wq[:, idx]), DIM).astype(bf16)
    bq_p = bq[idx].astype(np.float32).reshape(8, 128).T.copy()  # (128, 8)

    # permute Wo rows to match the flipped-PV attnT layout:
    # attnT row r = k*128 + p with k = 4gg + 2half + c, e = p//64, d = p%64,
    # head h = 4gg + 8half + 2c + e, original row h*64 + d.
    oidx = np.empty(DIM, dtype=np.int64)
    for k in range(8):
        gg, half, c = k // 4, (k % 4) // 2, k % 2
        for p in range(128):
            e, d = p // 64, p % 64
            h = 4 * gg + 8 * half + 2 * c + e
            oidx[k * 128 + p] = h * D + d
    wo_p = fold8(np.ascontiguousarray(Wo[oidx, :]), DIM).astype(bf16)

    wk_p = fold8(np.ascontiguousarray(Wk), KVD).astype(bf16)
    wv_p = fold8(np.ascontiguousarray(Wv), KVD).astype(bf16)

    # band masks [128 keys, (half, i=4, c=128)]: mA keeps key r >= query c
    # (j=0), mB keeps r < c (j=2); identical across the 8 head-blocks.
    r = np.arange(128)[:, None]
    c = np.arange(128)[None, :]
    mA_p = np.tile((r >= c), (1, 8)).astype(bf16)
    mB_p = np.tile((r < c), (1, 8)).astype(bf16)

    in_maps = []
    for core in range(NCORES):
        b, qt = core // QT, core % QT
        lo, hi = qt * T - HW, qt * T + T + HW
        xs = np.zeros((TH, DIM), dtype=np.float32)
        s0, s1 = max(lo, 0), min(hi, S)
        xs[s0 - lo:s1 - lo] = x[b, s0:s1]
        ind = np.zeros(TH, dtype=np.float32)
        ind[s0 - lo:s1 - lo] = 1.0
        # bkm[p, t] = bk[p]*ind[t]; bvm[p, (u,g,d)] = bv[g*64+d]*ind[u*128+p]
        bkm = (bk.astype(np.float32)[:, None] * ind[None, :]).astype(bf16)
        bvm = (ind.reshape(NU, 128).T[:, :, None, None] *
               bv.astype(np.float32).reshape(1, 1, 2, D)).reshape(
                   128, NU * 2 * D).astype(bf16)
        in_maps.append({
            "xTp": fold8(np.ascontiguousarray(xs.T), TH).astype(bf16),
            "wqp": wq_p, "wkp": wk_p, "wvp": wv_p, "wop": wo_p,
            "bqc": bq_p, "bkmp": bkm, "bvmp": bvm,
            "mAp": mA_p, "mBp": mB_p,
        })
    return in_maps


def kernel(x, Wq, bq, Wk, bk, Wv, bv, Wo, bo):
    from concourse.bass_utils import run_bass_kernel_spmd

    x, Wq, bq, Wk, bk, Wv, bv, Wo, bo = (
        np.asarray(a, dtype=np.float32)
        for a in (x, Wq, bq, Wk, bk, Wv, bv, Wo, bo))
    nc = _build_nc()
    in_maps = _host_prep(x, Wq, bq, Wk, bk, Wv, bv, Wo, bo)
    res = run_bass_kernel_spmd(nc, in_maps, core_ids=list(range(NCORES)))
    out = np.empty((B, S, DIM), dtype=np.float32)
    for c in range(NCORES):
        b, qt = c // QT, c % QT
        out[b, qt * T:(qt + 1) * T] = res.results[c]["out"].astype(np.float32)
    out += bo  # output bias is purely additive after the last matmul
    return out


# revision 10
# speedup vs baseline: 1.0205x; 1.0205x over previous
"""Trainium2 Bass kernel for sliding-window GQA attention block (v4).

Reference computation (B=2, S=4096, DIM=1024, H=16 q-heads, KV=2 kv-heads,
D=64, W=256 window):
    q = x@Wq + bq ; k = x@Wk + bk ; v = x@Wv + bv        (GQA repeat kv x8)
    local attention: query t attends keys [t-128, t+128) (zero-padded edges,
    no 1/sqrt(d) scaling), softmax, out = probs@v
    y = out@Wo + bo
Sharding: 8 cores = batch(2) x seq-quarter(4); each core computes 1024 query
rows end-to-end from a 1280-row haloed x slice; bo added on host.

v4 structure (vs 146us v2 baseline):
  - epilogue constants are HOST inputs: bkm = outer(bk, ind), bvm =
    bv x ind, band masks mA/mB.  x halo columns are zero so projections
    are already zero there -> single tensor_add fuses bias + halo mask.
  - PSUM as [128,1024] bank-pairs; both kv-halves of a score j-chunk
    share a pair -> ONE exp ACTIVATE per chunk (the 352-cycle ACT fixed
    cost amortizes), one reciprocal per (mt,gg).
  - attnT k-tile order k = 4*gg + 2*half + c (host permutes Wo rows to
    match) makes the normalize a single 3D-AP STT per (mt,gg,e).
  - band masks on DVE (tensor_mul with host masks); gpsimd only issues
    DMA descriptors (its semaphore path is too slow for the p2 chain).
  - head: all projections (Q/K/V) run before the loop; Q m0-3 are
    emitted k-major so the PE streams behind the xT/wq DMAs, which are
    interleaved per-chunk across the three DMA queues.  The attention
    loop is then pure scores/PV/out-proj.
  - ACT exp table preloaded with a dummy activation at t=0.
"""

import functools
import numpy as np

B, S, DIM = 2, 4096, 1024
H, KV, D = 16, 2, 64
W, HW = 256, 128
NCORES = 8
QT = 4           # sequence quarters
T = S // QT      # 1024 query rows per core
TH = T + 2 * HW  # 1280 haloed rows
NU = TH // 128   # 10 key/value u-tiles
KVD = KV * D     # 128


@functools.lru_cache(maxsize=1)
def _build_nc():
    import concourse.bacc as bacc
    import concourse.tile as tile
    from concourse import mybir

    f32 = mybir.dt.float32
    bf16 = mybir.dt.bfloat16
    Exp = mybir.ActivationFunctionType.Exp
    Identity = mybir.ActivationFunctionType.Identity
    MUL = mybir.AluOpType.mult

    nc = bacc.Bacc("TRN2", target_bir_lowering=False, debug=False)

    xTp = nc.dram_tensor("xTp", [128, 8 * TH], bf16, kind="ExternalInput")
    wqp = nc.dram_tensor("wqp", [128, 8 * DIM], bf16, kind="ExternalInput")
    wkp = nc.dram_tensor("wkp", [128, 8 * KVD], bf16, kind="ExternalInput")
    wvp = nc.dram_tensor("wvp", [128, 8 * KVD], bf16, kind="ExternalInput")
    wop = nc.dram_tensor("wop", [128, 8 * DIM], bf16, kind="ExternalInput")
    bqc = nc.dram_tensor("bqc", [128, 8], f32, kind="ExternalInput")
    bkmp = nc.dram_tensor("bkmp", [128, TH], bf16, kind="ExternalInput")
    bvmp = nc.dram_tensor("bvmp", [128, TH], bf16, kind="ExternalInput")
    out = nc.dram_tensor("out", [T, DIM], bf16, kind="ExternalOutput")

    with tile.TileContext(nc) as tc:
        with tc.tile_pool(name="const", bufs=1) as const, \
             tc.tile_pool(name="w", bufs=1) as wpool, \
             tc.tile_pool(name="act", bufs=1) as actp, \
             tc.tile_pool(name="attn", bufs=2) as attnp, \
             tc.tile_pool(name="ps", bufs=2, space="PSUM") as ps:

            # ---- ACT exp-table preload at t=0 (overlaps the DMA phase)
            dum = const.tile([1, 2], f32, tag="dum")
            dumo = const.tile([1, 2], f32, tag="dumo")
            nc.vector.memset(dum, 0.0)
            nc.scalar.activation(out=dumo, in_=dum, func=Exp)

            # ---- SBUF tiles -----------------------------------------------
            xT_sb = wpool.tile([128, 8 * TH], bf16, tag="xT")
            wq_sb = wpool.tile([128, 8 * DIM], bf16, tag="wq")
            wk_sb = wpool.tile([128, 8 * KVD], bf16, tag="wk")
            wv_sb = wpool.tile([128, 8 * KVD], bf16, tag="wv")
            wo_sb = wpool.tile([128, 8 * DIM], bf16, tag="wo")
            bq_sb = const.tile([128, 8], f32, tag="bq")
            bkm_sb = const.tile([128, TH], bf16, tag="bkm")
            bvm_sb = const.tile([128, TH], bf16, tag="bvm")

            # ---- DMA schedule: wk/wv first (tiny, K/V never wait), then
            # xT/wq chunk-interleaved across the three DMA queues in the
            # order the k-major Q matmuls consume them; consts next; wo
            # (first needed ~45us in) last, split across queues.
            # sync and gpsimd queues run ~145GB/s, scalar only ~70GB/s:
            # xT/wq alternate on the two fast queues (chunk k of both lands
            # before chunk k+1), everything small on scalar, wo split last.
            sq, sc, gq = nc.sync, nc.scalar, nc.gpsimd
            # first chunk split so the k=0 matmuls start ~2us earlier
            sq.dma_start(out=xT_sb[:, 0:768], in_=xTp[:, 0:768])
            gq.dma_start(out=wq_sb[:, 0:512], in_=wqp[:, 0:512])
            sq.dma_start(out=xT_sb[:, 768:TH], in_=xTp[:, 768:TH])
            gq.dma_start(out=wq_sb[:, 512:DIM], in_=wqp[:, 512:DIM])
            for k in range(1, 8):
                a, b = (sq, gq) if k % 2 == 0 else (gq, sq)
                a.dma_start(out=xT_sb[:, k * TH:(k + 1) * TH],
                            in_=xTp[:, k * TH:(k + 1) * TH])
                b.dma_start(out=wq_sb[:, k * DIM:(k + 1) * DIM],
                            in_=wqp[:, k * DIM:(k + 1) * DIM])
            sc.dma_start(out=wk_sb, in_=wkp[:, :])
            sc.dma_start(out=wv_sb, in_=wvp[:, :])
            sc.dma_start(out=bq_sb, in_=bqc[:, :])
            sc.dma_start(out=bkm_sb, in_=bkmp[:, :])
            sc.dma_start(out=bvm_sb, in_=bvmp[:, :])
            for k in range(8):
                q = (sq, gq, sc, sq, gq, sc, sq, gq)[k]
                q.dma_start(out=wo_sb[:, k * DIM:(k + 1) * DIM],
                            in_=wop[:, k * DIM:(k + 1) * DIM])

            # ---- activations / attention SBUF -----------------------------
            qT_sb = [actp.tile([128, 4 * T], bf16, tag=f"qT{g}", name=f"qT{g}")
                     for g in range(2)]
            kT_sb = actp.tile([128, TH], bf16, tag="kT")
            v_sb = actp.tile([128, NU * 256], bf16, tag="V")
            v_view = v_sb.rearrange("p (u g c) -> p u g c", u=NU, g=2)
            nc.vector.memset(v_view[:, :, :, 0:64], 1.0)
            bvm_v = bvm_sb.rearrange("p (u g d) -> p u g d", u=NU, g=2)
            attnT = actp.tile([128, 8 * T], bf16, tag="attnT")
            attnT_v = attnT.rearrange("p (k t) -> p k t", k=8)
            qvs = [qT_sb[g].rearrange("p (i t) -> p i t", i=4) for g in range(2)]
            fz = nc.gpsimd.to_reg(0.0)

            # ---- Q projection: two m-tiles k-inner per call; the first two
            # calls' k-loops stream directly behind the xT/wq DMAs.  Bias
            # copy into qT: even m on ACT, odd m on DVE.
            def q_sub(ms, interleaved=False):
                prs = {m: ps.tile([128, 1024], f32, tag="SP", bufs=2,
                                  name=f"qp{m}") for m in ms}
                for k in range(8):
                    for m in ms:
                        for n in range(2):
                            nc.tensor.matmul(
                                out=prs[m][:, n * 512:(n + 1) * 512],
                                lhsT=wq_sb[:, k * DIM + m * 128:
                                           k * DIM + (m + 1) * 128],
                                rhs=xT_sb[:, k * TH + HW + n * 512:
                                          k * TH + HW + (n + 1) * 512],
                                start=(k == 0), stop=(k == 7))
                for m in ms:
                    dst = qT_sb[m // 4][:, (m % 4) * T:(m % 4) * T + 1024]
                    if m % 2 == 0:
                        nc.scalar.activation(out=dst, in_=prs[m],
                                             func=Identity,
                                             bias=bq_sb[:, m:m + 1], scale=1.0)
                    else:
                        nc.vector.tensor_scalar_add(out=dst, in0=prs[m],
                                                    scalar1=bq_sb[:, m:m + 1])

            # sg0+sg1 interleaved k-major: 4 PSUM pairs live, PE consumes
            # each (xT[k], wq[k]) chunk-pair with 16 matmuls as it lands.
            def q_head():
                prs = {m: ps.tile([128, 1024], f32,
                                  tag=("SP" if m < 2 else "OP"), bufs=2,
                                  name=f"qp{m}") for m in range(4)}
                for k in range(8):
                    for m in range(4):
                        for n in range(2):
                            nc.tensor.matmul(
                                out=prs[m][:, n * 512:(n + 1) * 512],
                                lhsT=wq_sb[:, k * DIM + m * 128:
                                           k * DIM + (m + 1) * 128],
                                rhs=xT_sb[:, k * TH + HW + n * 512:
                                          k * TH + HW + (n + 1) * 512],
                                start=(k == 0), stop=(k == 7))
                for m in range(4):
                    dst = qT_sb[0][:, m * T:m * T + 1024]
                    if m % 2 == 0:
                        nc.scalar.activation(out=dst, in_=prs[m],
                                             func=Identity,
                                             bias=bq_sb[:, m:m + 1], scale=1.0)
                    else:
                        nc.vector.tensor_scalar_add(out=dst, in0=prs[m],
                                                    scalar1=bq_sb[:, m:m + 1])

            def k_proj():
                kp01 = ps.tile([128, 1024], f32, tag="SP", bufs=2, name="kp01")
                kp2 = ps.tile([128, 1024], f32, tag="OP", bufs=2, name="kp2")
                for k in range(8):
                    for c in range(2):
                        nc.tensor.matmul(
                            out=kp01[:, c * 512:(c + 1) * 512],
                            lhsT=wk_sb[:, k * KVD:(k + 1) * KVD],
                            rhs=xT_sb[:, k * TH + c * 512:k * TH + (c + 1) * 512],
                            start=(k == 0), stop=(k == 7))
                    nc.tensor.matmul(
                        out=kp2[:, 0:256],
                        lhsT=wk_sb[:, k * KVD:(k + 1) * KVD],
                        rhs=xT_sb[:, k * TH + 1024:k * TH + 1280],
                        start=(k == 0), stop=(k == 7))
                nc.vector.tensor_add(out=kT_sb[:, 0:1024], in0=kp01,
                                     in1=bkm_sb[:, 0:1024])
                nc.vector.tensor_add(out=kT_sb[:, 1024:1280], in0=kp2[:, 0:256],
                                     in1=bkm_sb[:, 1024:1280])

            def v_proj(ut):
                vp = ps.tile([128, 1024], f32, tag="OP", bufs=2, name="vp")
                for k in range(8):
                    nc.tensor.matmul(
                        out=vp[:, 0:128],
                        lhsT=xT_sb[:, k * TH + ut * 128:k * TH + (ut + 1) * 128],
                        rhs=wv_sb[:, k * KVD:(k + 1) * KVD],
                        start=(k == 0), stop=(k == 7))
                nc.vector.tensor_add(
                    out=v_view[:, ut, :, 64:128],
                    in0=vp[:, 0:128].rearrange("p (g c) -> p g c", g=2),
                    in1=bvm_v[:, ut])

            # ---- scores j-chunk: both kv-halves into one PSUM pair, one
            # exp over [128,1024]; band mask (j=0/2) via DVE tensor_mul.
            def scores_pair(mt, gg, j):
                qcol = mt * 128
                sp = ps.tile([128, 1024], f32, tag="SP", bufs=2, name="sp")
                for h in range(2):
                    nc.tensor.matmul(
                        out=sp[:, h * 512:(h + 1) * 512],
                        lhsT=kT_sb[h * 64:(h + 1) * 64,
                                   qcol + j * 128:qcol + (j + 1) * 128],
                        rhs=qvs[gg][h * 64:(h + 1) * 64, :, qcol:qcol + 128],
                        start=True, stop=True,
                        tile_position=(64 * h, 0))
                p2 = attnp.tile([128, 1024], bf16, tag="p2", bufs=10, name="p2")
                nc.scalar.activation(out=p2, in_=sp, func=Exp)
                # band mask on gpsimd (idle in the loop; the reordered
                # schedule gives its ~2us latency 4-6us of slack before PV)
                if j != 1:
                    pv8 = p2.rearrange("p (g c) -> p g c", g=8)
                    if j == 0:
                        nc.gpsimd.affine_select(
                            out=pv8, in_=pv8,
                            compare_op=mybir.AluOpType.is_ge, fill=fz,
                            base=0, pattern=[[0, 8], [-1, 128]],
                            channel_multiplier=1)
                    else:
                        nc.gpsimd.affine_select(
                            out=pv8, in_=pv8,
                            compare_op=mybir.AluOpType.is_ge, fill=fz,
                            base=-1, pattern=[[0, 8], [1, 128]],
                            channel_multiplier=-1)
                return p2

            # ---- PV + normalize: probs@[1|V] per half into one output
            # pair; one recip [64,1024]; one STT per e writes the four
            # k'-tiles (k' = 4gg + 2h + c, contiguous) of attnT.
            def pv(mt, gg, p2s):
                qcol = mt * 128
                op = ps.tile([128, 1024], f32, tag="OP", bufs=2, name="op")
                for h in range(2):
                    for j in range(3):
                        nc.tensor.matmul(
                            out=op[:, h * 512:(h + 1) * 512],
                            lhsT=v_view[:, mt + j, h, :],
                            rhs=p2s[j][:, h * 512:(h + 1) * 512],
                            start=(j == 0), stop=(j == 2))
                rc = attnp.tile([64, 1024], f32, tag="rc", bufs=2, name="rc")
                nc.vector.reciprocal_approx_fast(out=rc, in_=op[0:64, :])
                # free dim of op[64:128] is (h, c, e, t); for fixed e the
                # (h, c) dims are stride 512/256 -> merge into one 4-wide
                # dim matching attnT k'-tiles 4gg..4gg+3 (stride T).
                num = op[64:128, :].rearrange("p (hc e t) -> p hc e t",
                                              hc=4, e=2)
                rcv = rc.rearrange("p (hc e t) -> p hc e t", hc=4, e=2)
                for e in range(2):
                    nc.vector.scalar_tensor_tensor(
                        out=attnT_v[64 * e:64 * e + 64, 4 * gg:4 * gg + 4,
                                    qcol:qcol + 128],
                        in0=num[:, :, e, :], scalar=1.0,
                        in1=rcv[:, :, e, :], op0=MUL, op1=MUL)

            # ---- out projection: one PSUM pair; gg0 k'-tiles (0-3) first
            # so the accumulation overlaps the second PV group's normalize.
            def oproj(mt):
                qcol = mt * 128
                o2 = ps.tile([128, 1024], f32, tag="OP", bufs=2, name="o2")
                for k in range(8):
                    for n in range(2):
                        nc.tensor.matmul(
                            out=o2[:, n * 512:(n + 1) * 512],
                            lhsT=attnT[:, k * T + qcol:k * T + qcol + 128],
                            rhs=wo_sb[:, k * DIM + n * 512:
                                      k * DIM + (n + 1) * 512],
                            start=(k == 0), stop=(k == 7))
                out_t = attnp.tile([128, DIM], bf16, tag="outt", bufs=2,
                                   name="out_t")
                if mt % 2 == 0:
                    nc.scalar.copy(out=out_t, in_=o2)
                else:
                    nc.vector.tensor_copy(out=out_t, in_=o2)
                nc.sync.dma_start(out=out[qcol:qcol + 128, :], in_=out_t)

            # ---- head: minimum projections for loop start: Q m0-3 k-major
            # behind the DMA stream, K, V 0-2.  Q m4-7 run inside iteration
            # 0 (before its gg1 scores); V 3-9 are per-iteration fillers.
            q_head()
            k_proj()
            for ut in range(3):
                v_proj(ut)

            # ---- attention loop: per iteration
            #   [sc(mt,0) x3] [fillers] [pv(mt-1,1)] [sc(mt,1) x3]
            #   [oproj(mt-1)] [pv(mt,0)]
            # so each gg's exp->mask->PV->recip->STT chain is covered by
            # 5-7us of independent PE work before its consumer runs.
            prev_p2g1 = None
            prev = None
            for mt in range(8):
                last = (mt == 7)
                if last:
                    g1 = [scores_pair(mt, 1, j) for j in range(3)]
                g0 = [scores_pair(mt, 0, j) for j in range(3)]
                if mt == 0:
                    q_sub((4, 5))
                    q_sub((6, 7))
                if 1 <= mt <= 7:
                    v_proj(mt + 2)
                if prev is not None:
                    pv(prev, 1, prev_p2g1)
                if not last:
                    g1 = [scores_pair(mt, 1, j) for j in range(3)]
                if prev is not None:
                    oproj(prev)
                pv(mt, 0, g0)
                prev_p2g1 = g1
                prev = mt
            # epilogue: first half of the last out-projection overlaps the
            # final PV chain; second half follows its normalize.
            qcol = prev * 128
            o2 = ps.tile([128, 1024], f32, tag="OP", bufs=2, name="o2f")
            for k in range(4):
                for n in range(2):
                    nc.tensor.matmul(
                        out=o2[:, n * 512:(n + 1) * 512],
                        lhsT=attnT[:, k * T + qcol:k * T + qcol + 128],
                        rhs=wo_sb[:, k * DIM + n * 512:k * DIM + (n + 1) * 512],
                        start=(k == 0), stop=False)
            pv(prev, 1, prev_p2g1)
            for k in range(4, 8):
                for n in range(2):
                    nc.tensor.matmul(
                        out=o2[:, n * 512:(n + 1) * 512],
                        lhsT=attnT[:, k * T + qcol:k * T + qcol + 128],
                        rhs=wo_sb[:, k * DIM + n * 512:k * DIM + (n + 1) * 512],
                        start=False, stop=(k == 7))
            out_t = attnp.tile([128, DIM], bf16, tag="outt", bufs=2,
                               name="out_tf")
            nc.scalar.copy(out=out_t[:, 0:512], in_=o2[:, 0:512])
            nc.sync.dma_start(out=out[qcol:qcol + 128, 0:512],
                              in_=out_t[:, 0:512])
            nc.vector.tensor_copy(out=out_t[:, 512:1024], in_=o2[:, 512:1024])
            nc.sync.dma_start(out=out[qcol:qcol + 128, 512:1024],
                              in_=out_t[:, 512:1024])

    nc.compile()
    return nc


def _host_prep(x, Wq, bq, Wk, bk, Wv, bv, Wo, bo):
    import ml_dtypes
    bf16 = ml_dtypes.bfloat16

    def fold8(a, width):
        # [1024, width] -> [128, 8*width] with chunk k at cols k*width
        return np.ascontiguousarray(
            a.reshape(8, 128, width).transpose(1, 0, 2).reshape(128, 8 * width))

    # permute Wq columns so qT m-tile holds head m on partitions 0-63 and
    # head m+8 on partitions 64-127 (row-packed score matmuls)
    idx = np.empty(DIM, dtype=np.int64)
    for m in range(8):
        for j in range(128):
            h = m if j < 64 else m + 8
            idx[m * 128 + j] = h * D + (j % 64)
    wq_p = fold8(np.ascontiguousarray(Wq[:, idx]), DIM).astype(bf16)
    bq_p = bq[idx].astype(np.float32).reshape(8, 128).T.copy()  # (128, 8)

    # permute Wo rows to match the flipped-PV attnT layout:
    # attnT row r = k*128 + p with k = 4gg + 2half + c, e = p//64, d = p%64,
    # head h = 4gg + 8half + 2c + e, original row h*64 + d.
    oidx = np.empty(DIM, dtype=np.int64)
    for k in range(8):
        gg, half, c = k // 4, (k % 4) // 2, k % 2
        for p in range(128):
            e, d = p // 64, p % 64
            h = 4 * gg + 8 * half + 2 * c + e
            oidx[k * 128 + p] = h * D + d
    wo_p = fold8(np.ascontiguousarray(Wo[oidx, :]), DIM).astype(bf16)

    wk_p = fold8(np.ascontiguousarray(Wk), KVD).astype(bf16)
    wv_p = fold8(np.ascontiguousarray(Wv), KVD).astype(bf16)


    in_maps = []
    for core in range(NCORES):
        b, qt = core // QT, core % QT
        lo, hi = qt * T - HW, qt * T + T + HW
        xs = np.zeros((TH, DIM), dtype=np.float32)
        s0, s1 = max(lo, 0), min(hi, S)
        xs[s0 - lo:s1 - lo] = x[b, s0:s1]
        ind = np.zeros(TH, dtype=np.float32)
        ind[s0 - lo:s1 - lo] = 1.0
        # bkm[p, t] = bk[p]*ind[t]; bvm[p, (u,g,d)] = bv[g*64+d]*ind[u*128+p]
        bkm = (bk.astype(np.float32)[:, None] * ind[None, :]).astype(bf16)
        bvm = (ind.reshape(NU, 128).T[:, :, None, None] *
               bv.astype(np.float32).reshape(1, 1, 2, D)).reshape(
                   128, NU * 2 * D).astype(bf16)
        in_maps.append({
            "xTp": fold8(np.ascontiguousarray(xs.T), TH).astype(bf16),
            "wqp": wq_p, "wkp": wk_p, "wvp": wv_p, "wop": wo_p,
            "bqc": bq_p, "bkmp": bkm, "bvmp": bvm,
        })
    return in_maps


def kernel(x, Wq, bq, Wk, bk, Wv, bv, Wo, bo):
    from concourse.bass_utils import run_bass_kernel_spmd

    x, Wq, bq, Wk, bk, Wv, bv, Wo, bo = (
        np.asarray(a, dtype=np.float32)
        for a in (x, Wq, bq, Wk, bk, Wv, bv, Wo, bo))
    nc = _build_nc()
    in_maps = _host_prep(x, Wq, bq, Wk, bk, Wv, bv, Wo, bo)
    res = run_bass_kernel_spmd(nc, in_maps, core_ids=list(range(NCORES)))
    out = np.empty((B, S, DIM), dtype=np.float32)
    for c in range(NCORES):
        b, qt = c // QT, c % QT
        out[b, qt * T:(qt + 1) * T] = res.results[c]["out"].astype(np.float32)
    out += bo  # output bias is purely additive after the last matmul
    return out


# revision 11
# speedup vs baseline: 1.0788x; 1.0572x over previous
"""Trainium2 Bass kernel for sliding-window GQA attention block (v2).

Reference computation (B=2, S=4096, DIM=1024, H=16 q-heads, KV=2 kv-heads,
D=64, W=256 window):
    q = x@Wq + bq ; k = x@Wk + bk ; v = x@Wv + bv        (GQA repeat kv x8)
    local attention: query t attends keys [t-128, t+128) (zero-padded edges,
    no 1/sqrt(d) scaling), softmax, out = probs@v
    y = out@Wo + bo

Sharding: 8 cores = batch(2) x seq-quarter(4). Each core computes 1024
query rows end-to-end (all 16 heads) from a 1280-row haloed x slice.
No cross-core communication; host pads/transposes/gathers; bo is added
on the host (purely additive after the last matmul).

v2 structure (vs baseline):
  - K/V/Q projections run k-chunk-outer so the PE starts as soon as the
    first xT/wq DMA chunk lands; K is computed directly transposed with
    wk stationary; biases fold into DVE scalar_tensor_tensor ops (no K=1
    bias matmuls).
  - scores computed transposed as before (keys on partitions, kv-halves
    row-packed via tile_position so the two K=64 matmuls overlap).
  - probs@V is FLIPPED: the stationary is [V(64) | ones(64)] so the
    output lands directly in attnT orientation (head-dim on partitions)
    with the softmax denominator replicated across partitions 64-127 --
    no PE transposes, no partition broadcasts. Normalization is a DVE
    reciprocal_approx_fast on the replicated denominator + fused
    multiply into attnT.
  - the middle score chunk (always fully in-window) skips the band-mask
    multiply entirely.
  - out projection accumulates in PSUM and DMAs PSUM->DRAM directly.
"""

import functools
import numpy as np

B, S, DIM = 2, 4096, 1024
H, KV, D = 16, 2, 64
W, HW = 256, 128
NCORES = 8
QT = 4           # sequence quarters
T = S // QT      # 1024 query rows per core
TH = T + 2 * HW  # 1280 haloed rows
NU = TH // 128   # 10 key/value u-tiles


@functools.lru_cache(maxsize=1)
def _build_nc():
    import concourse.bacc as bacc
    import concourse.tile as tile
    from concourse import mybir

    f32 = mybir.dt.float32
    bf16 = mybir.dt.bfloat16
    Exp = mybir.ActivationFunctionType.Exp
    Identity = mybir.ActivationFunctionType.Identity
    MUL = mybir.AluOpType.mult

    nc = bacc.Bacc("TRN2", target_bir_lowering=False, debug=False)

    xT = nc.dram_tensor("xT", [DIM, TH], bf16, kind="ExternalInput")
    wq = nc.dram_tensor("Wq", [DIM, DIM], bf16, kind="ExternalInput")
    wk = nc.dram_tensor("Wk", [DIM, KV * D], bf16, kind="ExternalInput")
    wv = nc.dram_tensor("Wv", [DIM, KV * D], bf16, kind="ExternalInput")
    wo = nc.dram_tensor("Wo", [DIM, DIM], bf16, kind="ExternalInput")
    bqc = nc.dram_tensor("bqc", [128, 8], f32, kind="ExternalInput")
    bkmp = nc.dram_tensor("bkmp", [128, TH], bf16, kind="ExternalInput")
    bvmp = nc.dram_tensor("bvmp", [128, TH], bf16, kind="ExternalInput")
    out = nc.dram_tensor("out", [T, DIM], bf16, kind="ExternalOutput")

    with tile.TileContext(nc) as tc:
        with tc.tile_pool(name="const", bufs=1) as const, \
             tc.tile_pool(name="w", bufs=1) as wpool, \
             tc.tile_pool(name="act", bufs=1) as actp, \
             tc.tile_pool(name="attn", bufs=2) as attnp, \
             tc.tile_pool(name="ps", bufs=2, space="PSUM") as ps:

            # ---- weight/activation loads ----------------------------------
            # descriptor generation (~650ns per dma_start) serializes per
            # sequencer queue, so the critical first chunks (xT[k], wq[k],
            # then wk) round-robin across three DMA-capable queues; wv/wo
            # and small consts follow.
            qs = [nc.sync, nc.scalar, nc.gpsimd]
            xT_sb, wq_sb, wk_sb, wv_sb, wo_sb = [], [], [], [], []
            for k in range(8):
                t_x = wpool.tile([128, TH], bf16, tag=f"xT{k}", name=f"xT{k}")
                qs[(2 * k) % 3].dma_start(out=t_x,
                                          in_=xT[k * 128:(k + 1) * 128, :])
                xT_sb.append(t_x)
                t_q = wpool.tile([128, DIM], bf16, tag=f"wq{k}", name=f"wq{k}")
                qs[(2 * k + 1) % 3].dma_start(out=t_q,
                                              in_=wq[k * 128:(k + 1) * 128, :])
                wq_sb.append(t_q)
            for k in range(8):
                t_k = wpool.tile([128, KV * D], bf16, tag=f"wk{k}", name=f"wk{k}")
                qs[k % 3].dma_start(out=t_k, in_=wk[k * 128:(k + 1) * 128, :])
                wk_sb.append(t_k)
            bq_sb = const.tile([128, 8], f32, tag="bq")
            nc.scalar.dma_start(out=bq_sb, in_=bqc[:, :])
            bkm_sb = const.tile([128, TH], bf16, tag="bkm")
            nc.sync.dma_start(out=bkm_sb, in_=bkmp[:, :])
            for k in range(8):
                t_v = wpool.tile([128, KV * D], bf16, tag=f"wv{k}", name=f"wv{k}")
                qs[k % 3].dma_start(out=t_v, in_=wv[k * 128:(k + 1) * 128, :])
                wv_sb.append(t_v)
                t_o = wpool.tile([128, DIM], bf16, tag=f"wo{k}", name=f"wo{k}")
                qs[(k + 1) % 3].dma_start(out=t_o,
                                          in_=wo[k * 128:(k + 1) * 128, :])
                wo_sb.append(t_o)
            bvm_sb = const.tile([128, TH], bf16, tag="bvm")
            nc.scalar.dma_start(out=bvm_sb, in_=bvmp[:, :])
            bvm_v = bvm_sb.rearrange("p (u g d) -> p u g d", u=NU, g=2)

            # 0/1 band masks, transposed orientation (key partition r, query
            # col c), full 1024 wide = 8 blocks of 128 (4 head-blocks per
            # kv-half). Chunk j=0 valid where r >= c; j=2 valid where r < c;
            # j=1 is always fully valid and is never masked. Built on the
            # (head-phase-idle) DVE so the gpsimd queue stays free for DMA.
            mA8 = const.tile([128, 512], bf16, tag="mA8")
            mB8 = const.tile([128, 512], bf16, tag="mB8")
            nc.gpsimd.memset(mA8, 1.0)
            nc.gpsimd.memset(mB8, 1.0)
            for blk in range(4):
                nc.gpsimd.affine_select(
                    out=mA8[:, blk * 128:(blk + 1) * 128],
                    in_=mA8[:, blk * 128:(blk + 1) * 128],
                    compare_op=mybir.AluOpType.is_ge,
                    fill=0.0, base=0, pattern=[[-1, 128]],
                    channel_multiplier=1)
                nc.gpsimd.affine_select(
                    out=mB8[:, blk * 128:(blk + 1) * 128],
                    in_=mB8[:, blk * 128:(blk + 1) * 128],
                    compare_op=mybir.AluOpType.is_ge,
                    fill=0.0, base=-1, pattern=[[1, 128]],
                    channel_multiplier=-1)

            # ---- Q projection: qT tile g holds heads (m, m+8) on partition
            # halves for m = 4g..4g+3 (column-permuted Wq does the packing).
            # k-chunk-outer in groups of 4 m so the PE consumes xT/wq DMA
            # chunks as they arrive.
            qT_sb = [actp.tile([128, 4 * T], bf16, tag=f"qT{g}", name=f"qT{g}")
                     for g in range(2)]

            def q_group(grp):                    # m in [4*grp, 4*grp+4)
                pa = [ps.tile([128, 512], f32, tag="A", bufs=8,
                              name=f"qA{grp}{i}") for i in range(8)]
                for k in range(8):
                    for mi in range(4):
                        m = 4 * grp + mi
                        for n in range(2):
                            nc.tensor.matmul(
                                out=pa[2 * mi + n],
                                lhsT=wq_sb[k][:, m * 128:(m + 1) * 128],
                                rhs=xT_sb[k][:, HW + n * 512: HW + (n + 1) * 512],
                                start=(k == 0), stop=(k == 7))
                for mi in range(4):
                    m = 4 * grp + mi
                    off = (m % 4) * T
                    for n in range(2):
                        dst = qT_sb[grp][:, off + n * 512:off + (n + 1) * 512]
                        if mi % 2 == 0:
                            nc.scalar.activation(
                                out=dst, in_=pa[2 * mi + n], func=Identity,
                                bias=bq_sb[:, m:m + 1], scale=1.0)
                        else:
                            nc.vector.tensor_scalar_add(
                                out=dst, in0=pa[2 * mi + n],
                                scalar1=bq_sb[:, m:m + 1])

            # ---- K projection, directly transposed (kv*64+d on partitions,
            # token on free). wk stationary, xT moving; bias-add and halo
            # zeroing fused into the DVE copy. Emitted between the two Q
            # groups so the PE has work while Q-grp0's ACT copies drain.
            kT_sb = actp.tile([128, TH], bf16, tag="kT")

            def k_proj():
                k_ps = [ps.tile([128, 512], f32, tag="A", bufs=8,
                                name=f"kp{c}") for c in range(3)]
                k_dst = [k_ps[0][:, :], k_ps[1][:, :], k_ps[2][:, 0:256]]
                k_w = [512, 512, 256]
                for k in range(8):
                    for c in range(3):
                        nc.tensor.matmul(
                            out=k_dst[c], lhsT=wk_sb[k],
                            rhs=xT_sb[k][:, c * 512:c * 512 + k_w[c]],
                            start=(k == 0), stop=(k == 7))
                for c in range(3):
                    nc.vector.tensor_add(
                        out=kT_sb[:, c * 512:c * 512 + k_w[c]],
                        in0=k_dst[c],
                        in1=bkm_sb[:, c * 512:c * 512 + k_w[c]])

            q_group(0)
            k_proj()
            q_group(1)

            # ---- V projection (keys on partitions). v_sb u-tile layout per
            # kv-half g: [ones (64) | V (64)]; the 64 ones columns make the
            # flipped probs@[1|V] matmul emit the softmax denominator
            # REPLICATED on output partitions 0-63 (base 0, required by
            # reciprocal_approx_fast). ut-outer / k-inner with one PSUM tile
            # per ut: interleaved accumulation groups must not share a PSUM
            # bank (start=True clears the whole bank).
            v_sb = actp.tile([128, NU * 256], bf16, tag="V")
            v_view = v_sb.rearrange("p (u g c) -> p u g c", u=NU, g=2)
            nc.vector.memset(v_view[:, :, :, 0:64], 1.0)

            def v_proj(ut):
                v_ps = ps.tile([128, 512], f32, tag="A", bufs=8, name="v_ps")
                for k in range(8):
                    nc.tensor.matmul(
                        out=v_ps[:, 0:128],
                        lhsT=xT_sb[k][:, ut * 128:(ut + 1) * 128],
                        rhs=wv_sb[k], start=(k == 0), stop=(k == 7))
                nc.vector.tensor_add(
                    out=v_view[:, ut, :, 64:128],
                    in0=v_ps[:, 0:128].rearrange("p (g c) -> p g c", g=2),
                    in1=bvm_v[:, ut])

            # head computes only the u-tiles qtiles 0-2 need; the rest are
            # emitted just-in-time inside the attention loop (3-qtile lead)
            # to shorten the head and give the PE gap-filling work.
            for ut in range(5):
                v_proj(ut)

            # ---- attention + out projection -------------------------------
            attnT = actp.tile([128, 8 * T], bf16, tag="attnT")
            attnT_v = attnT.rearrange("p (k t) -> p k t", k=8)
            qvs = [qT_sb[g].rearrange("p (i t) -> p i t", i=4) for g in range(2)]

            def scores_j(mt, gg, j):
                """One score j-chunk: 2 row-packed MMs + exp + band mask,
                one single-bank PSUM tile and one p2 tile per kv-half."""
                qcol = mt * 128
                p2s = []
                for half in range(2):
                    s1 = ps.tile([128, 512], f32, tag="A", bufs=8, name="s1")
                    nc.tensor.matmul(
                        out=s1,
                        lhsT=kT_sb[half * 64:(half + 1) * 64,
                                   qcol + j * 128:qcol + (j + 1) * 128],
                        rhs=qvs[gg][half * 64:(half + 1) * 64, :,
                                    qcol:qcol + 128],
                        start=True, stop=True,
                        tile_position=(64 * half, 0))
                    p2 = attnp.tile([128, 512], bf16, tag="p2", bufs=18,
                                    name="p2")
                    nc.scalar.activation(out=p2, in_=s1, func=Exp)
                    if j == 0:
                        nc.vector.tensor_mul(p2, p2, mA8)
                    elif j == 2:
                        nc.vector.tensor_mul(p2, p2, mB8)
                    p2s.append(p2)
                return p2s

            def pv(mt, gg, p2s):
                """Flipped probs@[V|ones]: output in attnT orientation with
                replicated denominators; normalize + scatter into attnT."""
                qcol = mt * 128
                for half in range(2):
                    o_ps = ps.tile([128, 512], f32, tag="A", bufs=8,
                                   name="o_ps")
                    for j in range(3):
                        nc.tensor.matmul(
                            out=o_ps,
                            lhsT=v_view[:, mt + j, half, :],
                            rhs=p2s[j][half],
                            start=(j == 0), stop=(j == 2))
                    rc = attnp.tile([64, 512], f32, tag="rc", bufs=4,
                                    name="rc")
                    nc.vector.reciprocal_approx_fast(out=rc,
                                                     in_=o_ps[0:64, :])
                    k0 = 2 * gg + 4 * half
                    num = o_ps[64:128, :].rearrange("p (c e t) -> p c e t",
                                                    c=2, e=2)
                    rcv = rc.rearrange("p (c e t) -> p c e t", c=2, e=2)
                    for e in range(2):
                        nc.vector.scalar_tensor_tensor(
                            out=attnT_v[64 * e:64 * e + 64, k0:k0 + 2,
                                        qcol:qcol + 128],
                            in0=num[:, :, e, :], scalar=1.0,
                            in1=rcv[:, :, e, :], op0=MUL, op1=MUL)

            def oproj(mt):
                qcol = mt * 128
                o2s = [ps.tile([128, 512], f32, tag="A", bufs=8, name="o2")
                       for _ in range(2)]
                # k-order follows PV-group completion order (gg0 writes
                # k-tiles 0,1,4,5; gg1 writes 2,3,6,7) so the tail qtile's
                # accumulation overlaps the second PV group's normalize
                for k in (0, 1, 4, 5, 2, 3, 6, 7):
                    for n in range(2):
                        nc.tensor.matmul(
                            out=o2s[n],
                            lhsT=attnT[:, k * T + qcol:k * T + qcol + 128],
                            rhs=wo_sb[k][:, n * 512:(n + 1) * 512],
                            start=(k == 0), stop=(k == 7))
                out_t = attnp.tile([128, DIM], bf16, tag="outt", bufs=2,
                                   name="out_t")
                nc.scalar.copy(out=out_t[:, 0:512], in_=o2s[0])
                nc.sync.dma_start(out=out[qcol:qcol + 128, 0:512],
                                  in_=out_t[:, 0:512])
                nc.vector.tensor_copy(out=out_t[:, 512:1024], in_=o2s[1])
                nc.sync.dma_start(out=out[qcol:qcol + 128, 512:1024],
                                  in_=out_t[:, 512:1024])

            # software-pipelined at j-chunk granularity: the PSUM score ring
            # (2 tiles) forces each score MM to wait for the exp two steps
            # back; the previous qtile's PV-gg1 and out-projection are
            # emitted between score steps so the in-order PE queue always
            # has dense matmul work while exps drain.
            prev_p2g1 = None
            prev = None
            for mt in range(8):
                last = (mt == 7)
                if last:
                    # final iteration: g1 scores first so their exps have
                    # maximal lead before the tail pv(7,1)/oproj(7) drain
                    g1 = [scores_j(mt, 1, j) for j in range(3)]
                g0 = [scores_j(mt, 0, 0), scores_j(mt, 0, 1)]
                if prev is not None:
                    pv(prev, 1, prev_p2g1)
                g0.append(scores_j(mt, 0, 2))
                if prev is not None:
                    oproj(prev)
                if not last:
                    g1 = [scores_j(mt, 1, j) for j in range(3)]
                # front-load the JIT V tiles into the first qtiles, where
                # the attention pipeline is still filling and the PE has gaps
                for ut in {0: (5, 6), 1: (7, 8), 2: (9,)}.get(mt, ()):
                    v_proj(ut)
                pv(mt, 0, g0)
                prev_p2g1 = g1
                prev = mt
            pv(prev, 1, prev_p2g1)
            oproj(prev)

    nc.compile()
    return nc


def _host_prep(x, Wq, bq, Wk, bk, Wv, bv, Wo, bo):
    import ml_dtypes
    bf16 = ml_dtypes.bfloat16

    # permute Wq columns so qT m-tile holds head m on partitions 0-63 and
    # head m+8 on partitions 64-127 (row-packed score matmuls)
    idx = np.empty(DIM, dtype=np.int64)
    for m in range(8):
        for j in range(128):
            h = m if j < 64 else m + 8
            idx[m * 128 + j] = h * D + (j % 64)
    wq_p = np.ascontiguousarray(Wq[:, idx]).astype(bf16)
    bq_p = bq[idx].astype(np.float32).reshape(8, 128).T.copy()  # (128, 8)

    # permute Wo rows to match the flipped-PV attnT layout:
    # attnT row r = k*128 + p with k = 2gg + 4half + c, e = p//64, d = p%64,
    # head h = 4gg + 8half + 2c + e, original row h*64 + d.
    oidx = np.empty(DIM, dtype=np.int64)
    for k in range(8):
        half, gg, c = k // 4, (k % 4) // 2, k % 2
        for p in range(128):
            e, d = p // 64, p % 64
            h = 4 * gg + 8 * half + 2 * c + e
            oidx[k * 128 + p] = h * D + d
    wo_p = np.ascontiguousarray(Wo[oidx, :]).astype(bf16)

    wk_b = np.ascontiguousarray(Wk).astype(bf16)
    wv_b = np.ascontiguousarray(Wv).astype(bf16)

    in_maps = []
    for c in range(NCORES):
        b, qt = c // QT, c % QT
        lo, hi = qt * T - HW, qt * T + T + HW
        xs = np.zeros((TH, DIM), dtype=np.float32)
        s0, s1 = max(lo, 0), min(hi, S)
        xs[s0 - lo:s1 - lo] = x[b, s0:s1]
        ind_f = np.zeros(TH, dtype=np.float32)
        ind_f[s0 - lo:s1 - lo] = 1.0
        bkm = (bk.astype(np.float32)[:, None] * ind_f[None, :]).astype(bf16)
        bvm = (ind_f.reshape(NU, 128).T[:, :, None, None] *
               bv.astype(np.float32).reshape(1, 1, 2, D)).reshape(
                   128, NU * 2 * D).astype(bf16)
        in_maps.append({
            "xT": np.ascontiguousarray(xs.T).astype(bf16),
            "Wq": wq_p, "Wk": wk_b, "Wv": wv_b, "Wo": wo_p,
            "bqc": bq_p, "bkmp": bkm, "bvmp": bvm,
        })
    return in_maps


def kernel(x, Wq, bq, Wk, bk, Wv, bv, Wo, bo):
    from concourse.bass_utils import run_bass_kernel_spmd

    x, Wq, bq, Wk, bk, Wv, bv, Wo, bo = (
        np.asarray(a, dtype=np.float32)
        for a in (x, Wq, bq, Wk, bk, Wv, bv, Wo, bo))
    nc = _build_nc()
    in_maps = _host_prep(x, Wq, bq, Wk, bk, Wv, bv, Wo, bo)
    res = run_bass_kernel_spmd(nc, in_maps, core_ids=list(range(NCORES)))
    out = np.empty((B, S, DIM), dtype=np.float32)
    for c in range(NCORES):
        b, qt = c // QT, c % QT
        out[b, qt * T:(qt + 1) * T] = res.results[c]["out"].astype(np.float32)
    out += bo  # output bias is purely additive after the last matmul
    return out



# revision 12
# speedup vs baseline: 1.1002x; 1.0198x over previous
"""Trainium2 Bass kernel for sliding-window GQA attention block (v2).

Reference computation (B=2, S=4096, DIM=1024, H=16 q-heads, KV=2 kv-heads,
D=64, W=256 window):
    q = x@Wq + bq ; k = x@Wk + bk ; v = x@Wv + bv        (GQA repeat kv x8)
    local attention: query t attends keys [t-128, t+128) (zero-padded edges,
    no 1/sqrt(d) scaling), softmax, out = probs@v
    y = out@Wo + bo

Sharding: 8 cores = batch(2) x seq-quarter(4). Each core computes 1024
query rows end-to-end (all 16 heads) from a 1280-row haloed x slice.
No cross-core communication; host pads/transposes/gathers; bo is added
on the host (purely additive after the last matmul).

v2 structure (vs baseline):
  - K/V/Q projections run k-chunk-outer so the PE starts as soon as the
    first xT/wq DMA chunk lands; K is computed directly transposed with
    wk stationary; biases fold into DVE scalar_tensor_tensor ops (no K=1
    bias matmuls).
  - scores computed transposed as before (keys on partitions, kv-halves
    row-packed via tile_position so the two K=64 matmuls overlap).
  - probs@V is FLIPPED: the stationary is [V(64) | ones(64)] so the
    output lands directly in attnT orientation (head-dim on partitions)
    with the softmax denominator replicated across partitions 64-127 --
    no PE transposes, no partition broadcasts. Normalization is a DVE
    reciprocal_approx_fast on the replicated denominator + fused
    multiply into attnT.
  - the middle score chunk (always fully in-window) skips the band-mask
    multiply entirely.
  - out projection accumulates in PSUM and DMAs PSUM->DRAM directly.
"""

import functools
import numpy as np

B, S, DIM = 2, 4096, 1024
H, KV, D = 16, 2, 64
W, HW = 256, 128
NCORES = 8
QT = 4           # sequence quarters
T = S // QT      # 1024 query rows per core
TH = T + 2 * HW  # 1280 haloed rows
NU = TH // 128   # 10 key/value u-tiles


@functools.lru_cache(maxsize=1)
def _build_nc():
    import concourse.bacc as bacc
    import concourse.tile as tile
    from concourse import mybir

    f32 = mybir.dt.float32
    bf16 = mybir.dt.bfloat16
    Exp = mybir.ActivationFunctionType.Exp
    Identity = mybir.ActivationFunctionType.Identity
    MUL = mybir.AluOpType.mult

    nc = bacc.Bacc("TRN2", target_bir_lowering=False, debug=False)

    xT = nc.dram_tensor("xT", [DIM, TH], bf16, kind="ExternalInput")
    wq = nc.dram_tensor("Wq", [DIM, DIM], bf16, kind="ExternalInput")
    wk = nc.dram_tensor("Wk", [DIM, KV * D], bf16, kind="ExternalInput")
    wv = nc.dram_tensor("Wv", [DIM, KV * D], bf16, kind="ExternalInput")
    wo = nc.dram_tensor("Wo", [DIM, DIM], bf16, kind="ExternalInput")
    bqc = nc.dram_tensor("bqc", [128, 8], f32, kind="ExternalInput")
    bkmp = nc.dram_tensor("bkmp", [128, TH], bf16, kind="ExternalInput")
    bvmp = nc.dram_tensor("bvmp", [128, TH], bf16, kind="ExternalInput")
    out = nc.dram_tensor("out", [T, DIM], bf16, kind="ExternalOutput")

    with tile.TileContext(nc) as tc:
        with tc.tile_pool(name="const", bufs=1) as const, \
             tc.tile_pool(name="w", bufs=1) as wpool, \
             tc.tile_pool(name="act", bufs=1) as actp, \
             tc.tile_pool(name="attn", bufs=2) as attnp, \
             tc.tile_pool(name="ps", bufs=2, space="PSUM") as ps:

            # ---- weight/activation loads ----------------------------------
            # descriptor generation (~650ns per dma_start) serializes per
            # sequencer queue, so the critical first chunks (xT[k], wq[k],
            # then wk) round-robin across three DMA-capable queues; wv/wo
            # and small consts follow.
            # measured per-queue DMA speed: sync/gpsimd ~145GB/s, scalar
            # ~70GB/s.  xT/wq chunk-interleave on the fast queues in Q-proj
            # consumption order; scalar gets only small/late tensors.
            sq, sc, gq = nc.sync, nc.scalar, nc.gpsimd
            xT_sb, wq_sb, wk_sb, wv_sb, wo_sb = [], [], [], [], []
            for k in range(8):
                t_x = wpool.tile([128, TH], bf16, tag=f"xT{k}", name=f"xT{k}")
                (sq if k % 2 == 0 else gq).dma_start(
                    out=t_x, in_=xT[k * 128:(k + 1) * 128, :])
                xT_sb.append(t_x)
                t_q = wpool.tile([128, DIM], bf16, tag=f"wq{k}", name=f"wq{k}")
                (gq if k % 2 == 0 else sq).dma_start(
                    out=t_q, in_=wq[k * 128:(k + 1) * 128, :])
                wq_sb.append(t_q)
            for k in range(8):
                t_k = wpool.tile([128, KV * D], bf16, tag=f"wk{k}", name=f"wk{k}")
                sc.dma_start(out=t_k, in_=wk[k * 128:(k + 1) * 128, :])
                wk_sb.append(t_k)
            for k in range(8):
                t_v = wpool.tile([128, KV * D], bf16, tag=f"wv{k}", name=f"wv{k}")
                sc.dma_start(out=t_v, in_=wv[k * 128:(k + 1) * 128, :])
                wv_sb.append(t_v)
            bq_sb = const.tile([128, 8], f32, tag="bq")
            sc.dma_start(out=bq_sb, in_=bqc[:, :])
            bkm_sb = const.tile([128, TH], bf16, tag="bkm")
            sc.dma_start(out=bkm_sb, in_=bkmp[:, :])
            bvm_sb = const.tile([128, TH], bf16, tag="bvm")
            sc.dma_start(out=bvm_sb, in_=bvmp[:, :])
            bvm_v = bvm_sb.rearrange("p (u g d) -> p u g d", u=NU, g=2)
            wo_q = (sq, gq, sc, gq, sq, sc, sq, gq)
            for k in range(8):
                t_o = wpool.tile([128, DIM], bf16, tag=f"wo{k}", name=f"wo{k}")
                wo_q[k].dma_start(out=t_o, in_=wo[k * 128:(k + 1) * 128, :])
                wo_sb.append(t_o)

            # 0/1 band masks, transposed orientation (key partition r, query
            # col c), full 1024 wide = 8 blocks of 128 (4 head-blocks per
            # kv-half). Chunk j=0 valid where r >= c; j=2 valid where r < c;
            # j=1 is always fully valid and is never masked. Built on the
            # (head-phase-idle) DVE so the gpsimd queue stays free for DMA.
            mA8 = const.tile([128, 512], bf16, tag="mA8")
            mB8 = const.tile([128, 512], bf16, tag="mB8")
            nc.gpsimd.memset(mA8, 1.0)
            nc.gpsimd.memset(mB8, 1.0)
            for blk in range(4):
                nc.gpsimd.affine_select(
                    out=mA8[:, blk * 128:(blk + 1) * 128],
                    in_=mA8[:, blk * 128:(blk + 1) * 128],
                    compare_op=mybir.AluOpType.is_ge,
                    fill=0.0, base=0, pattern=[[-1, 128]],
                    channel_multiplier=1)
                nc.gpsimd.affine_select(
                    out=mB8[:, blk * 128:(blk + 1) * 128],
                    in_=mB8[:, blk * 128:(blk + 1) * 128],
                    compare_op=mybir.AluOpType.is_ge,
                    fill=0.0, base=-1, pattern=[[1, 128]],
                    channel_multiplier=-1)

            # ---- Q projection: qT tile g holds heads (m, m+8) on partition
            # halves for m = 4g..4g+3 (column-permuted Wq does the packing).
            # k-chunk-outer in groups of 4 m so the PE consumes xT/wq DMA
            # chunks as they arrive.
            fz = nc.gpsimd.to_reg(0.0)
            qT_sb = [actp.tile([128, 4 * T], bf16, tag=f"qT{g}", name=f"qT{g}")
                     for g in range(2)]

            def q_group(grp):                    # m in [4*grp, 4*grp+4)
                pa = [ps.tile([128, 512], f32, tag="A", bufs=8,
                              name=f"qA{grp}{i}") for i in range(8)]
                for k in range(8):
                    for mi in range(4):
                        m = 4 * grp + mi
                        for n in range(2):
                            nc.tensor.matmul(
                                out=pa[2 * mi + n],
                                lhsT=wq_sb[k][:, m * 128:(m + 1) * 128],
                                rhs=xT_sb[k][:, HW + n * 512: HW + (n + 1) * 512],
                                start=(k == 0), stop=(k == 7))
                for mi in range(4):
                    m = 4 * grp + mi
                    off = (m % 4) * T
                    for n in range(2):
                        dst = qT_sb[grp][:, off + n * 512:off + (n + 1) * 512]
                        if mi % 2 == 0:
                            nc.scalar.activation(
                                out=dst, in_=pa[2 * mi + n], func=Identity,
                                bias=bq_sb[:, m:m + 1], scale=1.0)
                        else:
                            nc.vector.tensor_scalar_add(
                                out=dst, in0=pa[2 * mi + n],
                                scalar1=bq_sb[:, m:m + 1])

            # ---- K projection, directly transposed (kv*64+d on partitions,
            # token on free). wk stationary, xT moving; bias-add and halo
            # zeroing fused into the DVE copy. Emitted between the two Q
            # groups so the PE has work while Q-grp0's ACT copies drain.
            kT_sb = actp.tile([128, TH], bf16, tag="kT")

            def k_proj():
                k_ps = [ps.tile([128, 512], f32, tag="A", bufs=8,
                                name=f"kp{c}") for c in range(3)]
                k_dst = [k_ps[0][:, :], k_ps[1][:, :], k_ps[2][:, 0:256]]
                k_w = [512, 512, 256]
                for k in range(8):
                    for c in range(3):
                        nc.tensor.matmul(
                            out=k_dst[c], lhsT=wk_sb[k],
                            rhs=xT_sb[k][:, c * 512:c * 512 + k_w[c]],
                            start=(k == 0), stop=(k == 7))
                for c in range(3):
                    nc.vector.tensor_add(
                        out=kT_sb[:, c * 512:c * 512 + k_w[c]],
                        in0=k_dst[c],
                        in1=bkm_sb[:, c * 512:c * 512 + k_w[c]])

            q_group(0)
            k_proj()
            q_group(1)

            # ---- V projection (keys on partitions). v_sb u-tile layout per
            # kv-half g: [ones (64) | V (64)]; the 64 ones columns make the
            # flipped probs@[1|V] matmul emit the softmax denominator
            # REPLICATED on output partitions 0-63 (base 0, required by
            # reciprocal_approx_fast). ut-outer / k-inner with one PSUM tile
            # per ut: interleaved accumulation groups must not share a PSUM
            # bank (start=True clears the whole bank).
            v_sb = actp.tile([128, NU * 256], bf16, tag="V")
            v_view = v_sb.rearrange("p (u g c) -> p u g c", u=NU, g=2)
            nc.vector.memset(v_view[:, :, :, 0:64], 1.0)

            def v_proj(ut):
                v_ps = ps.tile([128, 512], f32, tag="A", bufs=8, name="v_ps")
                for k in range(8):
                    nc.tensor.matmul(
                        out=v_ps[:, 0:128],
                        lhsT=xT_sb[k][:, ut * 128:(ut + 1) * 128],
                        rhs=wv_sb[k], start=(k == 0), stop=(k == 7))
                nc.vector.tensor_add(
                    out=v_view[:, ut, :, 64:128],
                    in0=v_ps[:, 0:128].rearrange("p (g c) -> p g c", g=2),
                    in1=bvm_v[:, ut])

            # head computes only the u-tiles qtiles 0-2 need; the rest are
            # emitted just-in-time inside the attention loop (3-qtile lead)
            # to shorten the head and give the PE gap-filling work.
            for ut in range(5):
                v_proj(ut)

            # ---- attention + out projection -------------------------------
            attnT = actp.tile([128, 8 * T], bf16, tag="attnT")
            attnT_v = attnT.rearrange("p (k t) -> p k t", k=8)
            qvs = [qT_sb[g].rearrange("p (i t) -> p i t", i=4) for g in range(2)]

            def scores_j(mt, gg, j):
                """One score j-chunk: 2 row-packed MMs + exp + band mask,
                one single-bank PSUM tile and one p2 tile per kv-half."""
                qcol = mt * 128
                p2s = []
                for half in range(2):
                    s1 = ps.tile([128, 512], f32, tag="A", bufs=8, name="s1")
                    nc.tensor.matmul(
                        out=s1,
                        lhsT=kT_sb[half * 64:(half + 1) * 64,
                                   qcol + j * 128:qcol + (j + 1) * 128],
                        rhs=qvs[gg][half * 64:(half + 1) * 64, :,
                                    qcol:qcol + 128],
                        start=True, stop=True,
                        tile_position=(64 * half, 0))
                    p2 = attnp.tile([128, 512], bf16, tag="p2", bufs=18,
                                    name="p2")
                    nc.scalar.activation(out=p2, in_=s1, func=Exp)
                    # gg0 masks on DVE (pv(mt,0) follows ~2us later); gg1
                    # masks on the idle gpsimd (pv(mt,1) runs mid-next
                    # iteration, ~4us of slack for its ~2us latency)
                    if j != 1 and gg == 0:
                        nc.vector.tensor_mul(p2, p2, mA8 if j == 0 else mB8)
                    elif j != 1:
                        pv4 = p2.rearrange("p (g c) -> p g c", g=4)
                        if j == 0:
                            nc.gpsimd.affine_select(
                                out=pv4, in_=pv4,
                                compare_op=mybir.AluOpType.is_ge, fill=fz,
                                base=0, pattern=[[0, 4], [-1, 128]],
                                channel_multiplier=1)
                        else:
                            nc.gpsimd.affine_select(
                                out=pv4, in_=pv4,
                                compare_op=mybir.AluOpType.is_ge, fill=fz,
                                base=-1, pattern=[[0, 4], [1, 128]],
                                channel_multiplier=-1)
                    p2s.append(p2)
                return p2s

            def pv(mt, gg, p2s):
                """Flipped probs@[V|ones]: output in attnT orientation with
                replicated denominators; normalize + scatter into attnT."""
                qcol = mt * 128
                for half in range(2):
                    o_ps = ps.tile([128, 512], f32, tag="A", bufs=8,
                                   name="o_ps")
                    for j in range(3):
                        nc.tensor.matmul(
                            out=o_ps,
                            lhsT=v_view[:, mt + j, half, :],
                            rhs=p2s[j][half],
                            start=(j == 0), stop=(j == 2))
                    rc = attnp.tile([64, 512], f32, tag="rc", bufs=4,
                                    name="rc")
                    nc.vector.reciprocal_approx_fast(out=rc,
                                                     in_=o_ps[0:64, :])
                    k0 = 2 * gg + 4 * half
                    num = o_ps[64:128, :].rearrange("p (c e t) -> p c e t",
                                                    c=2, e=2)
                    rcv = rc.rearrange("p (c e t) -> p c e t", c=2, e=2)
                    for e in range(2):
                        nc.vector.scalar_tensor_tensor(
                            out=attnT_v[64 * e:64 * e + 64, k0:k0 + 2,
                                        qcol:qcol + 128],
                            in0=num[:, :, e, :], scalar=1.0,
                            in1=rcv[:, :, e, :], op0=MUL, op1=MUL)

            def oproj(mt, last=False):
                qcol = mt * 128
                o2s = [ps.tile([128, 512], f32, tag="A", bufs=8, name="o2")
                       for _ in range(2)]
                # k-order follows PV-group completion order (gg0 writes
                # k-tiles 0,1,4,5; gg1 writes 2,3,6,7) so the tail qtile's
                # accumulation overlaps the second PV group's normalize
                for k in (0, 1, 4, 5, 2, 3, 6, 7):
                    for n in range(2):
                        nc.tensor.matmul(
                            out=o2s[n],
                            lhsT=attnT[:, k * T + qcol:k * T + qcol + 128],
                            rhs=wo_sb[k][:, n * 512:(n + 1) * 512],
                            start=(k == 0), stop=(k == 7))
                out_t = attnp.tile([128, DIM], bf16, tag="outt", bufs=2,
                                   name="out_t")
                if last:
                    nc.scalar.copy(out=out_t[:, 0:512], in_=o2s[0])
                else:
                    nc.vector.tensor_copy(out=out_t[:, 0:512], in_=o2s[0])
                nc.sync.dma_start(out=out[qcol:qcol + 128, 0:512],
                                  in_=out_t[:, 0:512])
                nc.vector.tensor_copy(out=out_t[:, 512:1024], in_=o2s[1])
                nc.sync.dma_start(out=out[qcol:qcol + 128, 512:1024],
                                  in_=out_t[:, 512:1024])

            # software-pipelined at j-chunk granularity: the PSUM score ring
            # (2 tiles) forces each score MM to wait for the exp two steps
            # back; the previous qtile's PV-gg1 and out-projection are
            # emitted between score steps so the in-order PE queue always
            # has dense matmul work while exps drain.
            prev_p2g1 = None
            prev = None
            for mt in range(8):
                last = (mt == 7)
                if last:
                    # final iteration: g1 scores first so their exps have
                    # maximal lead before the tail pv(7,1)/oproj(7) drain
                    g1 = [scores_j(mt, 1, j) for j in range(3)]
                g0 = [scores_j(mt, 0, 0), scores_j(mt, 0, 1)]
                if prev is not None:
                    pv(prev, 1, prev_p2g1)
                g0.append(scores_j(mt, 0, 2))
                if prev is not None:
                    oproj(prev)
                if not last:
                    g1 = [scores_j(mt, 1, j) for j in range(3)]
                # front-load the JIT V tiles into the first qtiles, where
                # the attention pipeline is still filling and the PE has gaps
                for ut in {0: (5, 6), 1: (7, 8), 2: (9,)}.get(mt, ()):
                    v_proj(ut)
                pv(mt, 0, g0)
                prev_p2g1 = g1
                prev = mt
            pv(prev, 1, prev_p2g1)
            oproj(prev, last=True)

    nc.compile()
    return nc


def _host_prep(x, Wq, bq, Wk, bk, Wv, bv, Wo, bo):
    import ml_dtypes
    bf16 = ml_dtypes.bfloat16

    # permute Wq columns so qT m-tile holds head m on partitions 0-63 and
    # head m+8 on partitions 64-127 (row-packed score matmuls)
    idx = np.empty(DIM, dtype=np.int64)
    for m in range(8):
        for j in range(128):
            h = m if j < 64 else m + 8
            idx[m * 128 + j] = h * D + (j % 64)
    wq_p = np.ascontiguousarray(Wq[:, idx]).astype(bf16)
    bq_p = bq[idx].astype(np.float32).reshape(8, 128).T.copy()  # (128, 8)

    # permute Wo rows to match the flipped-PV attnT layout:
    # attnT row r = k*128 + p with k = 2gg + 4half + c, e = p//64, d = p%64,
    # head h = 4gg + 8half + 2c + e, original row h*64 + d.
    oidx = np.empty(DIM, dtype=np.int64)
    for k in range(8):
        half, gg, c = k // 4, (k % 4) // 2, k % 2
        for p in range(128):
            e, d = p // 64, p % 64
            h = 4 * gg + 8 * half + 2 * c + e
            oidx[k * 128 + p] = h * D + d
    wo_p = np.ascontiguousarray(Wo[oidx, :]).astype(bf16)

    wk_b = np.ascontiguousarray(Wk).astype(bf16)
    wv_b = np.ascontiguousarray(Wv).astype(bf16)

    in_maps = []
    for c in range(NCORES):
        b, qt = c // QT, c % QT
        lo, hi = qt * T - HW, qt * T + T + HW
        xs = np.zeros((TH, DIM), dtype=np.float32)
        s0, s1 = max(lo, 0), min(hi, S)
        xs[s0 - lo:s1 - lo] = x[b, s0:s1]
        ind_f = np.zeros(TH, dtype=np.float32)
        ind_f[s0 - lo:s1 - lo] = 1.0
        bkm = (bk.astype(np.float32)[:, None] * ind_f[None, :]).astype(bf16)
        bvm = (ind_f.reshape(NU, 128).T[:, :, None, None] *
               bv.astype(np.float32).reshape(1, 1, 2, D)).reshape(
                   128, NU * 2 * D).astype(bf16)
        in_maps.append({
            "xT": np.ascontiguousarray(xs.T).astype(bf16),
            "Wq": wq_p, "Wk": wk_b, "Wv": wv_b, "Wo": wo_p,
            "bqc": bq_p, "bkmp": bkm, "bvmp": bvm,
        })
    return in_maps


def kernel(x, Wq, bq, Wk, bk, Wv, bv, Wo, bo):
    from concourse.bass_utils import run_bass_kernel_spmd

    x, Wq, bq, Wk, bk, Wv, bv, Wo, bo = (
        np.asarray(a, dtype=np.float32)
        for a in (x, Wq, bq, Wk, bk, Wv, bv, Wo, bo))
    nc = _build_nc()
    in_maps = _host_prep(x, Wq, bq, Wk, bk, Wv, bv, Wo, bo)
    res = run_bass_kernel_spmd(nc, in_maps, core_ids=list(range(NCORES)))
    out = np.empty((B, S, DIM), dtype=np.float32)
    for c in range(NCORES):
        b, qt = c // QT, c % QT
        out[b, qt * T:(qt + 1) * T] = res.results[c]["out"].astype(np.float32)
    out += bo  # output bias is purely additive after the last matmul
    return out

